# revision 37
# baseline (speedup 1.0000x reference)
"""Causal self-attention (B=2, N=2048, D=768, H=12, HD=64) on 8 TRN2 NeuronCores.

Sharding: tensor-parallel over (batch, head). Core c handles batch b = c//4 and
heads [3*(c%4), 3*(c%4)+3). Each core computes its 3 heads' attention plus the
matching 192 columns of the output projection (row-parallel W_proj), returning a
partial [2048, 768] output. Host sums the 4 partials per batch element and adds
b_proj.

Per-core kernel layout (all matmul operands bf16, accumulation fp32):
  - x arrives transposed (xT [768, 2048] bf16); the KQV projection produces a
    combined qkt1 [128, N] tile (q rows 0:64, k rows 64:128) with one bias-add,
    plus a swapped copy qkt2 ([k; q]) via two cheap bf16 SBUF copies so score
    matmuls for even/odd k-tiles run concurrently in separate PE row groups
    (row tiling; on HW the pair streams in ~half the time, sim-neutral).
  - scores are pre-transposed, S_T[k, q]; even/odd k-tile pairs write the two
    banks of one [128, 2, 512] PSUM tile, and a single paired exp (scale=1/8)
    covers both banks, halving ACT per-instruction overhead.  The softmax
    denominator is a matmul reduction: v is padded with a ones column and
    P_T feeds sa_T[d, q] in one accumulation chain per span.
  - causal structure: fully-masked (k > q) blocks are skipped; diagonal blocks
    are column-trimmed and the remaining triangle is masked multiplicatively
    with static bf16 [128, 512] masks (4x DVE mode).
  - the attention k-loop is software-pipelined by one pair (scores of pair
    i+1 issue before PV of pair i) so the in-order PE rides over the exp
    latency; v-projection and output-projection blocks are slotted between
    PV pairs of the neighboring attention phase (side queues).
  - normalization: reciprocal of the denominator row (DVE), partition-
    broadcast (GpSimd), sa_T scale (DVE) written straight into a stacked
    sa2 [128, 2, N] tile (h0 rows 0:64 / h1 rows 64:128 / h2 in slot 1,
    duplicated to both halves once per span).
  - output projection per 128-token tile and 384-feature half: one K=128
    matmul over the stacked h0/h1 pair plus a K=64 head-2 matmul accumulated
    into the same PSUM bank; the two feature halves put head-2 in different
    PE row groups so they overlap on HW.
  - a few dependency-free warm-up matmuls run during the initial DMA wait to
    start the PE p-state/HAM ramp early.
"""

import numpy as np
import ml_dtypes

import concourse.bass as bass
import concourse.mybir as mybir
import concourse.tile as tile
from concourse import bacc
from concourse.alu_op_type import AluOpType
from concourse.bass_utils import run_bass_kernel_spmd

F32 = mybir.dt.float32
BF16 = mybir.dt.bfloat16
AF = mybir.ActivationFunctionType
NP_BF16 = ml_dtypes.bfloat16

B, N, D = 2, 2048, 768
H, HD = 12, 64
HEADS_PER_CORE = 3
N_CORES = 8
NT = N // 128          # 16 token tiles of 128
NS = N // 512          # 4 query spans of 512
DC = D // 128          # 6 contraction chunks of 128

FLAGS = {
    "rt_st": True,     # row-tile even/odd score matmuls (HW concurrency)
    "pair_exp": True,  # one exp instruction per 2-bank score pair
    "gp_mask": False,  # causal mask on GpSimd instead of DVE
    "h2pair": True,    # head-2 proj matmuls in opposite row groups per e2-half
    "softpipe": True,  # emit scores of pair i+1 before PV of pair i (PE in-order)
    "side_work": True, # interleave v-proj/out-proj blocks between PV pairs
    "act_copies": False, # alternate proj-out copies between DVE and ACT
    "merge_out": False,  # one [128,768] out DMA per token tile instead of two halves
    "qk_side": False,    # emit qk projection of next head as side work
    "split_first_exp": False,  # per-bank exp on each span's first pair
    "hd_interleave": False,    # interleave heads 0/1 attention at pair granularity
    "swdge_loads": False,      # bulk x/weight loads via GpSimd SWDGE queue
    "tail_act": False,         # final-span proj copies alternate DVE/ACT
    "tail_pool_dma": False,    # final-span out DMAs alternate onto Pool SWDGE
}

_CACHE = {}


def build():
    nc = bacc.Bacc("TRN2", target_bir_lowering=False, debug=False)

    xT_d = nc.dram_tensor("xT", [D, N], BF16, kind="ExternalInput").ap()
    wqk_d = nc.dram_tensor("wqk", [128, HEADS_PER_CORE, DC * 128], BF16, kind="ExternalInput").ap()
    wv_d = nc.dram_tensor("wv", [128, DC * 192], BF16, kind="ExternalInput").ap()
    wp_d = nc.dram_tensor("wp", [128, 2, D], BF16, kind="ExternalInput").ap()
    bqk_d = nc.dram_tensor("bqk", [128, HEADS_PER_CORE], F32, kind="ExternalInput").ap()
    bvb_d = nc.dram_tensor("bvb", [128, 192], F32, kind="ExternalInput").ap()
    mask_d = nc.dram_tensor("masks", [128, 4, 512], BF16, kind="ExternalInput").ap()
    out_d = nc.dram_tensor("out", [N, D], F32, kind="ExternalOutput").ap()

    with tile.TileContext(nc) as tc, \
         nc.allow_low_precision(reason="bf16 matmul operands; accumulation stays fp32"):
        with tc.tile_pool(name="cn", bufs=1) as cn, \
             tc.tile_pool(name="qk", bufs=2) as qkp, \
             tc.tile_pool(name="pt", bufs=FLAGS.get("ptp_bufs", 8)) as ptp, \
             tc.tile_pool(name="sm", bufs=FLAGS.get("smp_bufs", 4)) as smp, \
             tc.tile_pool(name="ot", bufs=FLAGS.get("otp_bufs", 12)) as otp, \
             tc.tile_pool(name="psS", bufs=2, space="PSUM") as psS, \
             tc.tile_pool(name="psP", bufs=2, space="PSUM") as psP, \
             tc.tile_pool(name="psM", bufs=2, space="PSUM") as psM:

            # ---- constant loads ----
            # head-0 qk proj needs x span 0 + its wqk slice first; spread DMAs
            # over both HWDGE queues (SP + ACT) so they land in parallel.
            x_sp = [cn.tile([128, DC, 512], BF16, name=f"x_sp{s}") for s in range(NS)]
            xr = xT_d.rearrange("(c p) n -> p c n", p=128)
            wqk_sb = cn.tile([128, HEADS_PER_CORE, DC, 128], BF16, name="wqk_sb")
            bqk_sb = cn.tile([128, HEADS_PER_CORE], F32, name="bqk_sb")
            wv_sb = cn.tile([128, DC, 192], BF16, name="wv_sb")
            bvb_sb = cn.tile([128, 192], F32, name="bvb_sb")
            wp_sb = cn.tile([128, 2, D], BF16, name="wp_sb")
            # DMA order follows first-use time: x spans early, wp last.
            nc.sync.dma_start(x_sp[0][:, 0:3, :], xr[:, 0:3, 0:512])
            nc.scalar.dma_start(wqk_sb[:, 0, :, :], wqk_d[:, 0, :].rearrange("p (c m) -> p c m", c=DC))
            nc.scalar.dma_start(x_sp[0][:, 3:6, :], xr[:, 3:6, 0:512])
            nc.sync.dma_start(bqk_sb[:], bqk_d)
            nc.sync.dma_start(x_sp[1][:, 0:3, :], xr[:, 0:3, 512:1024])
            nc.scalar.dma_start(x_sp[1][:, 3:6, :], xr[:, 3:6, 512:1024])
            nc.sync.dma_start(wv_sb[:], wv_d.rearrange("p (c m) -> p c m", c=DC))
            if not FLAGS["gp_mask"]:
                mask_sb = cn.tile([128, 4, 512], BF16, name="mask_sb")
                nc.scalar.dma_start(mask_sb[:], mask_d)
            nc.scalar.dma_start(bvb_sb[:], bvb_d)
            for s in range(2, NS):
                xsl = xr[:, :, s * 512:(s + 1) * 512]
                nc.sync.dma_start(x_sp[s][:, 0:3, :], xsl[:, 0:3, :])
                nc.scalar.dma_start(x_sp[s][:, 3:6, :], xsl[:, 3:6, :])
            nc.sync.dma_start(wqk_sb[:, 1:3, :, :], wqk_d[:, 1:3, :].rearrange("p h (c m) -> p h c m", c=DC))
            nc.scalar.dma_start(wp_sb[:], wp_d)

            vf = cn.tile([128, NT, HEADS_PER_CORE, 65], BF16, name="vf")
            sa2 = cn.tile([128, 2, N], BF16, name="sa2")

            # warm up the PE ramp during the initial DMA wait: a few dependency-
            # free matmuls start the p-state clock (and nudge HAM) early.
            warm_sb = cn.tile([1, 512], BF16, name="warm_sb")
            nc.vector.memset(warm_sb[:], 1.0)
            warm_ps = psM.tile([128, 512], F32, name="warm_ps", tag="misc")
            for _ in range(6):
                nc.tensor.matmul(warm_ps[0:1, :], warm_sb[0:1, 0:1],
                                 warm_sb[0:1, :], start=True, stop=True)

            def alloc_qk():
                qkt1 = qkp.tile([128, N], BF16, name="qkt1")
                qkt2 = qkp.tile([128, N], BF16, name="qkt2") if FLAGS["rt_st"] else None
                return qkt1, qkt2

            def emit_qk_span(j, s, tiles):
                qkt1, qkt2 = tiles
                qk_ps = psM.tile([128, 512], F32, name="qk_ps", tag="misc")
                for c in range(DC):
                    nc.tensor.matmul(
                        qk_ps[:],
                        wqk_sb[:, j, c, :],
                        x_sp[s][:, c, :],
                        start=(c == 0), stop=(c == DC - 1),
                    )
                sl = slice(s * 512, (s + 1) * 512)
                nc.vector.tensor_scalar_add(qkt1[:, sl], qk_ps[:], bqk_sb[:, j:j + 1])
                if FLAGS["rt_st"]:
                    nc.vector.tensor_copy(qkt2[0:64, sl], qkt1[64:128, sl])
                    nc.vector.tensor_copy(qkt2[64:128, sl], qkt1[0:64, sl])

            def emit_qk(j):
                tiles = alloc_qk()
                for s in range(NS):
                    emit_qk_span(j, s, tiles)
                return tiles

            def emit_v(nt_range):
                # V projection for all 3 heads fused: v[n, o], o in [0, 192)
                for nt in nt_range:
                    v_ps = psM.tile([128, 512], F32, name="v_ps", tag="misc")
                    for c in range(DC):
                        nc.tensor.matmul(
                            v_ps[:, 0:192],
                            x_sp[nt // 4][:, c, (nt % 4) * 128:(nt % 4 + 1) * 128],
                            wv_sb[:, c, :],
                            start=(c == 0), stop=(c == DC - 1),
                        )
                    nc.vector.tensor_tensor(
                        vf[:, nt, :, 0:64],
                        v_ps[:, 0:192].rearrange("p (h d) -> p h d", h=3),
                        bvb_sb[:].rearrange("p (h d) -> p h d", h=3),
                        op=mybir.AluOpType.add,
                    )
                    nc.vector.memset(vf[:, nt, :, 64:65], 1.0)

            from collections import deque
            side_q = deque()   # small thunks slotted between PV pairs
            big_q = deque()    # big thunks slotted into span-start exp-wait slack

            def side():
                if side_q:
                    side_q.popleft()()

            def big_side():
                if big_q:
                    big_q.popleft()()

            def flush_side():
                while big_q:
                    big_q.popleft()()
                while side_q:
                    side_q.popleft()()

            def emit_sc_exp(j, qkt1, qkt2, s, pair, split_exp=False):
                offs = [max(0, (kt - 4 * s)) * 128 for kt in pair]
                sc_ps = psS.tile([128, 2, 512], F32, name="sc_ps", tag="sc")
                pt = ptp.tile([128, 2, 512], BF16, name="pt")
                for idx, ktile in enumerate(pair):
                    off = offs[idx]
                    ksl = slice(ktile * 128, (ktile + 1) * 128)
                    qsl = slice(s * 512 + off, (s + 1) * 512)
                    if FLAGS["rt_st"] and idx == 1:
                        lhsT, rhs, tp = qkt1[64:128, ksl], qkt2[64:128, qsl], (64, 0)
                    elif FLAGS["rt_st"]:
                        lhsT, rhs, tp = qkt2[0:64, ksl], qkt1[0:64, qsl], (0, 0)
                    else:
                        lhsT, rhs, tp = qkt1[64:128, ksl], qkt1[0:64, qsl], (0, 0)
                    nc.tensor.matmul(
                        sc_ps[:, idx, off:512], lhsT, rhs,
                        start=True, stop=True, tile_position=tp,
                    )
                    if split_exp:
                        off_ = offs[idx]
                        nc.scalar.activation(pt[:, idx, off_:512],
                                             sc_ps[:, idx, off_:512],
                                             AF.Exp, scale=0.125)
                if split_exp:
                    pass
                elif FLAGS["pair_exp"]:
                    o = min(offs)
                    nc.scalar.activation(pt[:, :, o:512], sc_ps[:, :, o:512],
                                         AF.Exp, scale=0.125)
                else:
                    for idx in range(2):
                        off = offs[idx]
                        nc.scalar.activation(pt[:, idx, off:512],
                                             sc_ps[:, idx, off:512],
                                             AF.Exp, scale=0.125)

                for idx, ktile in enumerate(pair):
                    off = offs[idx]
                    if ktile >= 4 * s:
                        jj = ktile - 4 * s
                        if FLAGS["gp_mask"]:
                            nc.gpsimd.affine_select(
                                out=pt[:, idx, off:512],
                                in_=pt[:, idx, off:512],
                                compare_op=AluOpType.is_ge, fill=0.0,
                                base=off - jj * 128,
                                pattern=[[1, 512 - off]],
                                channel_multiplier=-1,
                            )
                        else:
                            nc.vector.tensor_tensor(
                                pt[:, idx, off:512], pt[:, idx, off:512],
                                mask_sb[:, jj, off:512],
                                op=mybir.AluOpType.mult,
                            )
                return pt, offs

            def attn_span_units(j, qkt1, qkt2, s):
                """Generator: one yield per emitted PV pair; norm chain at end."""
                lo_j = 64 if j == 1 else 0
                hi_j = lo_j + 64
                slot_j = 0 if j < 2 else 1
                nkt = 4 * s + 4
                pv_ps = psP.tile([65, 512], F32, name="pv_ps", tag="pv")

                def emit_pv(pair, pt, offs):
                    for idx, ktile in enumerate(pair):
                        off = offs[idx]
                        nc.tensor.matmul(
                            pv_ps[:, off:512],
                            vf[:, ktile, j, :],
                            pt[:, idx, off:512],
                            start=(ktile == 0), stop=(ktile == nkt - 1),
                        )

                pending = None
                for kt0 in range(0, nkt, 2):
                    pair = (kt0, kt0 + 1)
                    cur = emit_sc_exp(
                        j, qkt1, qkt2, s, pair,
                        split_exp=(FLAGS["split_first_exp"] and kt0 == 0))
                    if kt0 == 0:
                        big_side()
                    if FLAGS["softpipe"]:
                        if pending is not None:
                            emit_pv(*pending)
                            yield
                        pending = (pair, *cur)
                    else:
                        emit_pv(pair, *cur)
                        yield
                if pending is not None:
                    emit_pv(*pending)
                sl = slice(s * 512, (s + 1) * 512)
                rc = smp.tile([1, 512], F32, name="rc")
                nc.vector.reciprocal(rc[:], pv_ps[64:65, :])
                rb = smp.tile([64, 512], F32, name="rb")
                nc.gpsimd.partition_broadcast(rb[:], rc[:])
                nc.vector.tensor_tensor(
                    sa2[lo_j:hi_j, slot_j, sl], pv_ps[0:64, :], rb[:],
                    op=mybir.AluOpType.mult,
                )
                if j == 2:
                    nc.vector.tensor_copy(sa2[64:128, 1, sl], sa2[0:64, 1, sl])

            def emit_attn(j, qkt1, qkt2, s_range):
                for s in s_range:
                    for _ in attn_span_units(j, qkt1, qkt2, s):
                        side()

            ot_tiles = {}

            def emit_proj_block(nt, e2, tail=False):
                ntl = slice(nt * 128, (nt + 1) * 128)
                esl = slice(e2 * 384, (e2 + 1) * 384)
                pX = psM.tile([128, 512], F32, name="pX", tag="misc")
                nc.tensor.matmul(
                    pX[:, 0:384], sa2[:, 0, ntl], wp_sb[:, 0, esl],
                    start=True, stop=False,
                )
                lo = 64 * e2 if FLAGS["h2pair"] else 0
                nc.tensor.matmul(
                    pX[:, 0:384], sa2[lo:lo + 64, 1, ntl],
                    wp_sb[lo:lo + 64, 1, esl],
                    start=False, stop=True, tile_position=(lo, 0),
                )
                if FLAGS["merge_out"]:
                    if e2 == 0:
                        ot_tiles[nt] = otp.tile([128, D], F32, name="ot")
                    ot = ot_tiles[nt]
                    osl = esl
                else:
                    ot = otp.tile([128, 384], F32, name="ot")
                    osl = slice(0, 384)
                use_act = (FLAGS["act_copies"] or (tail and FLAGS["tail_act"])) \
                    and (nt + e2) % 2 == 1
                if use_act:
                    nc.scalar.activation(ot[:, osl], pX[:, 0:384], AF.Copy)
                else:
                    nc.vector.tensor_copy(ot[:, osl], pX[:, 0:384])
                if tail and FLAGS["tail_pool_dma"] and nt % 2 == 1:
                    eng = nc.gpsimd
                else:
                    eng = nc.sync if (nt % 2 == 0) else nc.scalar
                if FLAGS["merge_out"]:
                    if e2 == 1:
                        eng.dma_start(out_d[ntl, :], ot[:])
                        del ot_tiles[nt]
                else:
                    eng.dma_start(out_d[ntl, esl], ot[:, 0:384])

            if FLAGS["hd_interleave"]:
                qkt1_0, qkt2_0 = emit_qk(0)
                qkt1_1, qkt2_1 = emit_qk(1)
                emit_v(range(0, 4))
                for s in range(NS):
                    if s < NS - 1 and FLAGS["side_work"]:
                        side_q.extend(
                            (lambda nt=nt: emit_v([nt]))
                            for nt in range(4 * (s + 1), 4 * (s + 1) + 4))
                    g0 = attn_span_units(0, qkt1_0, qkt2_0, s)
                    g1 = attn_span_units(1, qkt1_1, qkt2_1, s)
                    alive = [g0, g1]
                    while alive:
                        for g in list(alive):
                            try:
                                next(g)
                            except StopIteration:
                                alive.remove(g)
                        side()
                    flush_side()
            else:
                tiles0 = alloc_qk()
                emit_qk_span(0, 0, tiles0)
                qkt1_0, qkt2_0 = tiles0
                emit_v(range(0, 4))
                tiles1 = alloc_qk() if FLAGS["qk_side"] else None
                for s in range(NS):
                    if s < NS - 1 and FLAGS["side_work"]:
                        side_q.append(
                            lambda ss=s + 1: emit_qk_span(0, ss, tiles0))
                        side_q.extend(
                            (lambda nt=nt: emit_v([nt]))
                            for nt in range(4 * (s + 1), 4 * (s + 1) + 4))
                    elif s < NS - 1:
                        emit_attn(0, qkt1_0, qkt2_0, [s])
                        emit_qk_span(0, s + 1, tiles0)
                        emit_v(range(4 * (s + 1), 4 * (s + 1) + 4))
                        continue
                    if FLAGS["qk_side"] and s == NS - 1:
                        side_q.extend(
                            (lambda ss=ss: emit_qk_span(1, ss, tiles1))
                            for ss in range(NS))
                    emit_attn(0, qkt1_0, qkt2_0, [s])
                    flush_side()
                if FLAGS["qk_side"]:
                    qkt1_1, qkt2_1 = tiles1
                    tiles2 = alloc_qk()
                    side_q.extend(
                        (lambda ss=ss: emit_qk_span(2, ss, tiles2)) for ss in range(NS))
                else:
                    qkt1_1, qkt2_1 = emit_qk(1)
                emit_attn(1, qkt1_1, qkt2_1, range(NS))
                flush_side()
            if FLAGS["qk_side"] and not FLAGS["hd_interleave"]:
                qkt1_2, qkt2_2 = tiles2
            else:
                qkt1_2, qkt2_2 = emit_qk(2)
            for s in range(NS):
                emit_attn(2, qkt1_2, qkt2_2, [s])
                tail = (s == NS - 1)
                if FLAGS["side_work"]:
                    side_q.extend(
                        (lambda nt=nt, e2=e2, tail=tail: emit_proj_block(nt, e2, tail))
                        for nt in range(4 * s, 4 * s + 4) for e2 in range(2))
                    if s == NS - 1:
                        flush_side()
                else:
                    for nt in range(4 * s, 4 * s + 4):
                        for e2 in range(2):
                            emit_proj_block(nt, e2, tail)
            flush_side()

    nc.compile()
    return nc


def _host_shard(x, W_kqv, b_kqv, W_proj, b_proj):
    """Build the 8 per-core input maps."""
    masks = np.zeros((128, 4, 512), dtype=NP_BF16)
    yy = np.arange(512)[None, :]
    xx = np.arange(128)[:, None]
    for jj in range(4):
        masks[:, jj, :] = (yy >= xx + jj * 128).astype(NP_BF16)

    in_maps = []
    for c in range(N_CORES):
        b = c // 4
        h0 = (c % 4) * HEADS_PER_CORE
        hs = [h0, h0 + 1, h0 + 2]
        xT = np.ascontiguousarray(x[b].T).astype(NP_BF16)        # [768, 2048]

        wqk = np.empty((128, HEADS_PER_CORE, DC, 128), dtype=NP_BF16)  # reshaped below
        bqk = np.empty((128, HEADS_PER_CORE), dtype=np.float32)
        for j, h in enumerate(hs):
            # rows 0:64 = q features, 64:128 = k features
            wj = np.concatenate([W_kqv[h, 64:128], W_kqv[h, 0:64]], axis=0)  # [128, 768]
            # wqk[p, j, c, m] = wj[m, c*128+p]
            wqk[:, j, :, :] = wj.T.reshape(DC, 128, 128).transpose(1, 0, 2).astype(NP_BF16)
            bqk[:, j] = np.concatenate([b_kqv[h, 64:128], b_kqv[h, 0:64]])

        wv_all = np.zeros((D, 192), dtype=np.float32)
        for j, h in enumerate(hs):
            wv_all[:, j * 64:(j + 1) * 64] = W_kqv[h, 128:192].T
        wv = np.ascontiguousarray(
            wv_all.reshape(DC, 128, 192).transpose(1, 0, 2)).astype(NP_BF16)

        # stacked proj weights: slot 0 = [h0; h1], slot 1 = [h2; h2]
        wp = np.empty((128, 2, D), dtype=NP_BF16)
        wp[0:64, 0, :] = W_proj[:, hs[0] * 64:(hs[0] + 1) * 64].T.astype(NP_BF16)
        wp[64:128, 0, :] = W_proj[:, hs[1] * 64:(hs[1] + 1) * 64].T.astype(NP_BF16)
        wp[0:64, 1, :] = W_proj[:, hs[2] * 64:(hs[2] + 1) * 64].T.astype(NP_BF16)
        wp[64:128, 1, :] = wp[0:64, 1, :]

        bvb = np.tile(np.concatenate([b_kqv[h, 128:192] for h in hs])[None, :],
                      (128, 1)).astype(np.float32)

        in_maps.append({
            "xT": xT, "wqk": wqk.reshape(128, HEADS_PER_CORE, DC * 128),
            "wv": wv.reshape(128, DC * 192), "wp": wp,
            "bqk": bqk, "bvb": bvb, "masks": masks,
        })
    return in_maps


def profile(inputs):
    """Run once with NTFF tracing; return real HW exec_time_ns (max over cores)."""
    if "nc" not in _CACHE:
        _CACHE["nc"] = build()
    nc = _CACHE["nc"]
    in_maps = _host_shard(**inputs)
    try:
        res = run_bass_kernel_spmd(nc, in_maps, list(range(N_CORES)), trace=True)
        return res.exec_time_ns
    except Exception:
        return None


def kernel(x, W_kqv, b_kqv, W_proj, b_proj):
    x = np.asarray(x, dtype=np.float32)
    W_kqv = np.asarray(W_kqv, dtype=np.float32)
    b_kqv = np.asarray(b_kqv, dtype=np.float32)
    W_proj = np.asarray(W_proj, dtype=np.float32)
    b_proj = np.asarray(b_proj, dtype=np.float32)

    if "nc" not in _CACHE:
        _CACHE["nc"] = build()
    nc = _CACHE["nc"]

    in_maps = _host_shard(x, W_kqv, b_kqv, W_proj, b_proj)
    res = run_bass_kernel_spmd(nc, in_maps, list(range(N_CORES)))

    out = np.empty((B, N, D), dtype=np.float32)
    for b in range(B):
        acc = res.results[4 * b]["out"].astype(np.float32)
        for c in range(4 * b + 1, 4 * b + 4):
            acc = acc + res.results[c]["out"]
        out[b] = acc + b_proj[None, :]
    return out
